# revision 22
# baseline (speedup 1.0000x reference)
"""Block-sparse attention (ViT-style block-causal) on 8 Trainium2 NeuronCores.

Strategy: data-parallel over batch (4 batches per core), SPMD, no collectives.

Math (per batch, tokens pre-permuted on host into block-sorted order so the
mask is block-causal with 16-token blocks):
  - qkv projection computed directly in transposed layouts:
      QT/KT [o, t] = wqkT.T @ xT   (o on partitions, 2 heads per 128-tile)
      V     [t, o] = xT.T  @ wvT   (natural layout, + ones column appended)
  - S^T[k, q] = KT.T @ QT per head (contraction over head dim = 64)
  - P^T = exp(scale * S^T)  (no max-subtraction; scores bounded ~|6.5|)
    multiplied by a 16-granular staircase 0/1 mask on the diagonal 128-tile
  - A^T_unnorm[d, q] (+ denom row) = V_aug.T @ P^T   (augmented-ones trick:
    row 64 of the output accumulates sum_k P^T = softmax denominator)
  - denominators collected per batch -> reciprocal -> broadcast across
    partitions via a K=12 selector matmul -> A^T = A^T_unnorm * recip
  - y[t, o] = A^T.T @ wpT  (A^T layout feeds proj directly, transpose-free)

All matmul operands bf16 (fp32 PSUM accumulation); output fp32.

Engine assignment: PE matmuls; ACT exclusively Exp (avoids activation-table
reloads); DVE all PSUM-reading copies + normalize multiplies + reciprocal;
GpSimd the SBUF-only staircase-mask multiplies. Batches are software-
pipelined: batch b's qkv/attention overlaps batch b-1's normalize/proj.
"""

import numpy as np
import ml_dtypes

B, N, C, H = 32, 576, 768, 12
HD = C // H                      # 64
CORES = 8
BL = B // CORES                  # 4 batches per core
T = BL * N                       # 2304 tokens per core
GRID, BS = 24, 4
SCALE = HD ** -0.5
CA = C // 128                    # 6 contraction tiles
BF16 = ml_dtypes.bfloat16

# q-chunks of the S^T matmul for each key tile kk: (q_offset, width)
S_CHUNKS = {
    0: [(0, 512), (512, 64)],
    1: [(128, 448)],
    2: [(256, 320)],
    3: [(384, 192)],
    4: [(512, 64)],
}
KSZ = [128, 128, 128, 128, 64]   # key-tile sizes (576 = 4*128 + 64)
TSZ = KSZ                        # token-tile sizes within a batch

TRACE = False
LAST_EXEC_NS = None
LAST_TRACE_PATH = None

_prog_cache = {}


def _block_perm():
    r = np.arange(GRID) // BS
    bi = (r[:, None] * (GRID // BS) + r[None, :]).reshape(-1)
    return np.argsort(bi, kind="stable")


def _build_program(have_qkb=False):
    from contextlib import ExitStack

    import concourse.mybir as mybir
    import concourse.tile as tile
    from concourse import bacc

    dt = mybir.dt
    f32 = dt.float32
    bf16 = dt.bfloat16
    mult = mybir.AluOpType.mult

    nc = bacc.Bacc("TRN2", target_bir_lowering=False, debug=False,
                   num_devices=CORES)

    xT_d = nc.dram_tensor("xT", [C, T], bf16, kind="ExternalInput").ap()
    wqk_d = nc.dram_tensor("wqkT", [C, 2 * C], bf16, kind="ExternalInput").ap()
    wv_d = nc.dram_tensor("wvT", [C, C], bf16, kind="ExternalInput").ap()
    wp_d = nc.dram_tensor("wpT", [C, C], bf16, kind="ExternalInput").ap()
    mask_d = nc.dram_tensor("maskT", [128, 128], bf16, kind="ExternalInput").ap()
    E_d = nc.dram_tensor("E", [2, 128], bf16, kind="ExternalInput").ap()
    y_d = nc.dram_tensor("y", [T, C], f32, kind="ExternalOutput").ap()
    if have_qkb:
        qkb_d = nc.dram_tensor("qkb", [128, 12], f32, kind="ExternalInput").ap()

    with tile.TileContext(nc) as tc, ExitStack() as ctx:
        consts = ctx.enter_context(tc.tile_pool(name="consts", bufs=1))
        xt_pool = ctx.enter_context(tc.tile_pool(name="xt", bufs=2))
        qkt_pool = ctx.enter_context(tc.tile_pool(name="qkt", bufs=2))
        v_pool = ctx.enter_context(tc.tile_pool(name="v", bufs=2))
        pt_pool = ctx.enter_context(tc.tile_pool(name="pt", bufs=12))
        atu_pool = ctx.enter_context(tc.tile_pool(name="atu", bufs=6))
        at_pool = ctx.enter_context(tc.tile_pool(name="at", bufs=2))
        d_pool = ctx.enter_context(tc.tile_pool(name="d", bufs=8))
        y_pool = ctx.enter_context(tc.tile_pool(name="y", bufs=3))

        ps_mm = ctx.enter_context(tc.tile_pool(name="ps_mm", bufs=3, space="PSUM"))
        ps_s = ctx.enter_context(tc.tile_pool(name="ps_s", bufs=2, space="PSUM"))
        ps_b = ctx.enter_context(tc.tile_pool(name="ps_b", bufs=1, space="PSUM"))
        ps_av = ctx.enter_context(tc.tile_pool(name="ps_av", bufs=2, space="PSUM"))

        # ---- constants / weights (per-contraction-tile tiles so the first
        # matmuls only wait on the first DMA) ----
        wqk = [consts.tile([128, 2 * C], bf16, tag=f"wqk{a}", name=f"wqk{a}") for a in range(CA)]
        wv = [consts.tile([128, C], bf16, tag=f"wv{a}", name=f"wv{a}") for a in range(CA)]
        wp = [consts.tile([128, C], bf16, tag=f"wp{a}", name=f"wp{a}") for a in range(CA)]
        for a in range(CA):
            nc.sync.dma_start(out=wqk[a], in_=wqk_d[128 * a:128 * (a + 1), :])
        for a in range(CA):
            nc.sync.dma_start(out=wv[a], in_=wv_d[128 * a:128 * (a + 1), :])
        for a in range(CA):
            nc.sync.dma_start(out=wp[a], in_=wp_d[128 * a:128 * (a + 1), :])
        maskT = consts.tile([128, 128], bf16, tag="mask")
        nc.sync.dma_start(out=maskT, in_=mask_d[:, :])
        E_sb = consts.tile([2, 128], bf16, tag="E")
        nc.sync.dma_start(out=E_sb, in_=E_d[:, :])
        if have_qkb:
            qkb = consts.tile([128, 12], f32, tag="qkb")
            nc.sync.dma_start(out=qkb, in_=qkb_d[:, :])

        def emit_qkv(b):
            """Load x^T and produce QT/KT [o, t], V [t, o]+ones for batch b."""
            xt = [xt_pool.tile([128, N], bf16, tag=f"xt{a}", name=f"xt{a}") for a in range(CA)]
            for a in range(CA):
                nc.sync.dma_start(
                    out=xt[a],
                    in_=xT_d[128 * a:128 * (a + 1), N * b:N * (b + 1)],
                )
            qkt = qkt_pool.tile([128, 12, N], bf16, tag="qkt")
            for ot in range(12):
                for qc in (0, 288):
                    ps = ps_mm.tile([128, 512], f32, tag="mm")
                    for a in range(CA):
                        nc.tensor.matmul(
                            ps[:, :288],
                            lhsT=wqk[a][:, 128 * ot:128 * (ot + 1)],
                            rhs=xt[a][:, qc:qc + 288],
                            start=(a == 0), stop=(a == CA - 1),
                        )
                    if have_qkb:
                        nc.scalar.activation(
                            out=qkt[:, ot, qc:qc + 288], in_=ps[:, :288],
                            func=mybir.ActivationFunctionType.Identity,
                            bias=qkb[:, ot:ot + 1],
                        )
                    else:
                        nc.vector.tensor_copy(out=qkt[:, ot, qc:qc + 288],
                                              in_=ps[:, :288])
            v = v_pool.tile([128, 5, H, HD + 1], bf16, tag="v")
            nc.vector.memset(v[:, :, :, HD:HD + 1], 1.0)
            for kk in range(5):
                tsz = TSZ[kk]
                for ch in range(2):
                    ps = ps_mm.tile([128, 512], f32, tag="mm")
                    for a in range(CA):
                        nc.tensor.matmul(
                            ps[:tsz, :384],
                            lhsT=xt[a][:, 128 * kk:128 * kk + tsz],
                            rhs=wv[a][:, 384 * ch:384 * (ch + 1)],
                            start=(a == 0), stop=(a == CA - 1),
                        )
                    nc.vector.tensor_copy(
                        out=v[0:tsz, kk, 6 * ch:6 * (ch + 1), 0:HD],
                        in_=ps[:tsz, :384],
                    )
            return qkt, v

        def emit_pts(qkt, h):
            """S^T -> exp -> staircase mask for one head, returns 5 P^T tiles."""
            po = 64 * (h % 2)
            j = h // 2
            QT = qkt[po:po + 64, j, :]
            KT = qkt[po:po + 64, 6 + j, :]
            pts = []
            for kk in range(5):
                ksz = KSZ[kk]
                ko = 128 * kk
                pt = pt_pool.tile([128, N], bf16, tag="pt")
                for (qo, qw) in S_CHUNKS[kk]:
                    ps = ps_s.tile([128, 512], f32, tag="s")
                    nc.tensor.matmul(
                        ps[:ksz, :qw],
                        lhsT=KT[:, ko:ko + ksz],
                        rhs=QT[:, qo:qo + qw],
                        start=True, stop=True,
                    )
                    nc.scalar.activation(
                        out=pt[0:ksz, qo:qo + qw],
                        in_=ps[:ksz, :qw],
                        func=mybir.ActivationFunctionType.Exp,
                        scale=float(SCALE),
                    )
                # staircase mask on the diagonal tile (SBUF-only: GpSimd)
                nc.gpsimd.tensor_tensor(
                    out=pt[0:ksz, ko:ko + ksz],
                    in0=pt[0:ksz, ko:ko + ksz],
                    in1=maskT[0:ksz, 0:ksz],
                    op=mult,
                )
                pts.append(pt)
            return pts

        def emit_av(v, h, pts, dpair):
            """AV matmuls + unnormalized A^T staging; denom row into dpair."""
            atu = atu_pool.tile([HD + 1, N], f32, tag="atu")
            ps0 = ps_av.tile([HD + 1, 512], f32, tag="av")
            for kk in range(4):
                ko = 128 * kk
                nc.tensor.matmul(
                    ps0[:, ko:512],
                    lhsT=v[0:128, kk, h, :],
                    rhs=pts[kk][0:128, ko:512],
                    start=(kk == 0), stop=(kk == 3),
                )
            nc.vector.tensor_copy(out=atu[:, 0:512], in_=ps0[:, 0:512])
            ps1 = ps_av.tile([HD + 1, 512], f32, tag="av")
            for kk in range(5):
                ksz = KSZ[kk]
                nc.tensor.matmul(
                    ps1[:, 0:64],
                    lhsT=v[0:ksz, kk, h, :],
                    rhs=pts[kk][0:ksz, 512:576],
                    start=(kk == 0), stop=(kk == 4),
                )
            nc.vector.tensor_copy(out=atu[:, 512:576], in_=ps1[:, 0:64])
            nc.sync.dma_start(out=dpair[h % 2:h % 2 + 1, :],
                              in_=atu[HD:HD + 1, :])
            return atu

        def emit_pair_norm(at, j2, dpair, atu0, atu1):
            drecp = d_pool.tile([2, N], f32, tag="drecp")
            nc.vector.reciprocal_approx_fast(out=drecp, in_=dpair)
            drecp16 = d_pool.tile([2, N], bf16, tag="drecp16")
            nc.vector.tensor_copy(out=drecp16, in_=drecp)
            for (qo, qw) in ((0, 512), (512, 64)):
                psb = ps_b.tile([128, 512], f32, tag="b")
                nc.tensor.matmul(
                    psb[:, :qw],
                    lhsT=E_sb[0:2, :],
                    rhs=drecp16[:, qo:qo + qw],
                    start=True, stop=True,
                )
                for hh, atu in ((0, atu0), (1, atu1)):
                    nc.vector.tensor_tensor(
                        out=at[64 * hh:64 * hh + 64, j2, qo:qo + qw],
                        in0=atu[0:64, qo:qo + qw],
                        in1=psb[64 * hh:64 * hh + 64, :qw],
                        op=mult,
                    )

        def emit_attention(b, qkt, v, at):
            """All heads, software-pipelined: S/exp/mask of head h+1 is
            emitted before AV of head h so the PE's in-order stream never
            waits on the ACT->GpSimd chain; pair normalization follows the
            pair's second AV."""
            dpairs = [d_pool.tile([2, N], f32, tag="dpair", name=f"dpair{j}")
                      for j in range(CA)]
            pts_q = {}
            atus = {}
            for h in range(H):
                pts_q[h] = emit_pts(qkt, h)
                if h >= 1:
                    atus[h - 1] = emit_av(v, h - 1, pts_q.pop(h - 1),
                                          dpairs[(h - 1) // 2])
                if h >= 2 and h % 2 == 0:
                    j2 = h // 2 - 1
                    emit_pair_norm(at, j2, dpairs[j2],
                                   atus.pop(2 * j2), atus.pop(2 * j2 + 1))
            atus[H - 1] = emit_av(v, H - 1, pts_q.pop(H - 1), dpairs[CA - 1])
            emit_pair_norm(at, CA - 1, dpairs[CA - 1],
                           atus.pop(H - 2), atus.pop(H - 1))

        def emit_proj(b, at):
            for tt in range(5):
                tsz = TSZ[tt]
                to = 128 * tt
                ysb = y_pool.tile([128, C], f32, tag="y")
                for ch in range(2):
                    ps = ps_mm.tile([128, 512], f32, tag="mm")
                    for a in range(CA):
                        nc.tensor.matmul(
                            ps[:tsz, :384],
                            lhsT=at[:, a, to:to + tsz],
                            rhs=wp[a][:, 384 * ch:384 * (ch + 1)],
                            start=(a == 0), stop=(a == CA - 1),
                        )
                    nc.vector.tensor_copy(
                        out=ysb[0:tsz, 384 * ch:384 * (ch + 1)],
                        in_=ps[:tsz, :384],
                    )
                nc.sync.dma_start(
                    out=y_d[N * b + to:N * b + to + tsz, :],
                    in_=ysb[0:tsz, :],
                )

        # software-pipelined batch loop: batch b's producers are emitted
        # before batch b-1's proj so the PE always has dense work
        pending = None
        for b in range(BL):
            qkt, v = emit_qkv(b)
            if pending is not None:
                emit_proj(pending[0], pending[1])
            at = at_pool.tile([128, CA, N], bf16, tag="at")
            emit_attention(b, qkt, v, at)
            pending = (b, at)
        emit_proj(pending[0], pending[1])

    nc.compile()
    return nc


def _get_program(have_qkb=False):
    key = ("nc", have_qkb)
    if key not in _prog_cache:
        _prog_cache[key] = _build_program(have_qkb)
    return _prog_cache[key]


def kernel(x, qkv_w, qkv_b, proj_w, proj_b):
    global LAST_EXEC_NS, LAST_TRACE_PATH
    from concourse.bass_utils import run_bass_kernel_spmd

    x = np.asarray(x, np.float32)
    qkv_w = np.asarray(qkv_w, np.float32)
    qkv_b = np.asarray(qkv_b, np.float32)
    proj_w = np.asarray(proj_w, np.float32)
    proj_b = np.asarray(proj_b, np.float32)

    perm = _block_perm()
    x_s = x[:, perm, :]

    wqkT = np.ascontiguousarray(qkv_w[:2 * C].T).astype(BF16)
    wvT = np.ascontiguousarray(qkv_w[2 * C:].T).astype(BF16)
    wpT = np.ascontiguousarray(proj_w.T).astype(BF16)
    idx = np.arange(128)
    maskT = (idx[:, None] // 16 <= idx[None, :] // 16).astype(BF16)
    # E[i, m] broadcasts drecp16 row i across partitions [64i, 64i+64) when
    # used as lhsT of a K=2 matmul
    E = np.zeros((2, 128), BF16)
    E[0, :64] = 1
    E[1, 64:] = 1

    have_qkb = bool(np.any(qkv_b[:2 * C]))
    shared = {"wqkT": wqkT, "wvT": wvT, "wpT": wpT, "maskT": maskT, "E": E}
    if have_qkb:
        shared["qkb"] = np.ascontiguousarray(
            qkv_b[:2 * C].reshape(12, 128).T).astype(np.float32)
    in_maps = []
    for c in range(CORES):
        xb = x_s[BL * c:BL * (c + 1)]                        # [BL, N, C]
        xT = np.ascontiguousarray(xb.transpose(2, 0, 1).reshape(C, T)).astype(BF16)
        m = {"xT": xT}
        m.update(shared)
        in_maps.append(m)

    nc = _get_program(have_qkb)
    res = run_bass_kernel_spmd(nc, in_maps, core_ids=list(range(CORES)),
                               trace=TRACE)
    LAST_EXEC_NS = res.exec_time_ns
    LAST_TRACE_PATH = (res.instructions_and_trace[1]
                       if res.instructions_and_trace else None)

    y_s = np.empty((B, N, C), np.float32)
    for c in range(CORES):
        y_s[BL * c:BL * (c + 1)] = res.results[c]["y"].reshape(BL, N, C)
    y = np.empty_like(y_s)
    y[:, perm, :] = y_s
    # v-bias and proj-bias contribute a constant per-channel vector to every
    # token: fold them in exactly here (attention rows sum to 1).
    tail = proj_b.astype(np.float64) + qkv_b[2 * C:].astype(np.float64) @ proj_w.T.astype(np.float64)
    if np.any(tail):
        y += tail.astype(np.float32)[None, None, :]
    return y


# revision 23
# speedup vs baseline: 1.0261x; 1.0261x over previous
"""Block-sparse attention (ViT-style block-causal) on 8 Trainium2 NeuronCores.

Strategy: data-parallel over batch (4 batches per core), SPMD, no collectives.

Math (per batch, tokens pre-permuted on host into block-sorted order so the
mask is block-causal with 16-token blocks):
  - qkv projection computed directly in transposed layouts:
      QT/KT [o, t] = wqkT.T @ xT   (o on partitions, 2 heads per 128-tile)
      V     [t, o] = xT.T  @ wvT   (natural layout, + ones column appended)
  - S^T[k, q] = KT.T @ QT per head (contraction over head dim = 64)
  - P^T = exp(scale * S^T)  (no max-subtraction; scores bounded ~|6.5|)
    multiplied by a 16-granular staircase 0/1 mask on the diagonal 128-tile
  - A^T_unnorm[d, q] (+ denom row) = V_aug.T @ P^T   (augmented-ones trick:
    row 64 of the output accumulates sum_k P^T = softmax denominator)
  - denominators collected per batch -> reciprocal -> broadcast across
    partitions via a K=12 selector matmul -> A^T = A^T_unnorm * recip
  - y[t, o] = A^T.T @ wpT  (A^T layout feeds proj directly, transpose-free)

All matmul operands bf16 (fp32 PSUM accumulation); output fp32.

Engine assignment: PE matmuls; ACT exclusively Exp (avoids activation-table
reloads); DVE all PSUM-reading copies + normalize multiplies + reciprocal;
GpSimd the SBUF-only staircase-mask multiplies. Batches are software-
pipelined: batch b's qkv/attention overlaps batch b-1's normalize/proj.
"""

import numpy as np
import ml_dtypes

B, N, C, H = 32, 576, 768, 12
HD = C // H                      # 64
CORES = 8
BL = B // CORES                  # 4 batches per core
T = BL * N                       # 2304 tokens per core
GRID, BS = 24, 4
SCALE = HD ** -0.5
CA = C // 128                    # 6 contraction tiles
BF16 = ml_dtypes.bfloat16

# q-chunks of the S^T matmul for each key tile kk: (q_offset, width)
S_CHUNKS = {
    0: [(0, 512), (512, 64)],
    1: [(128, 448)],
    2: [(256, 320)],
    3: [(384, 192)],
    4: [(512, 64)],
}
KSZ = [128, 128, 128, 128, 64]   # key-tile sizes (576 = 4*128 + 64)
TSZ = KSZ                        # token-tile sizes within a batch

TRACE = False
LAST_EXEC_NS = None
LAST_TRACE_PATH = None

_prog_cache = {}


def _block_perm():
    r = np.arange(GRID) // BS
    bi = (r[:, None] * (GRID // BS) + r[None, :]).reshape(-1)
    return np.argsort(bi, kind="stable")


def _build_program(have_qkb=False):
    from contextlib import ExitStack

    import concourse.mybir as mybir
    import concourse.tile as tile
    from concourse import bacc

    dt = mybir.dt
    f32 = dt.float32
    bf16 = dt.bfloat16
    mult = mybir.AluOpType.mult

    nc = bacc.Bacc("TRN2", target_bir_lowering=False, debug=False,
                   num_devices=CORES)

    xT_d = nc.dram_tensor("xT", [C, T], bf16, kind="ExternalInput").ap()
    wqk_d = nc.dram_tensor("wqkT", [C, 2 * C], bf16, kind="ExternalInput").ap()
    wv_d = nc.dram_tensor("wvT", [C, C], bf16, kind="ExternalInput").ap()
    wp_d = nc.dram_tensor("wpT", [C, C], bf16, kind="ExternalInput").ap()
    mask_d = nc.dram_tensor("maskT", [128, 128], bf16, kind="ExternalInput").ap()
    E_d = nc.dram_tensor("E", [2, 128], bf16, kind="ExternalInput").ap()
    y_d = nc.dram_tensor("y", [T, C], f32, kind="ExternalOutput").ap()
    if have_qkb:
        qkb_d = nc.dram_tensor("qkb", [128, 12], f32, kind="ExternalInput").ap()

    with tile.TileContext(nc) as tc, ExitStack() as ctx:
        consts = ctx.enter_context(tc.tile_pool(name="consts", bufs=1))
        xt_pool = ctx.enter_context(tc.tile_pool(name="xt", bufs=2))
        qkt_pool = ctx.enter_context(tc.tile_pool(name="qkt", bufs=2))
        v_pool = ctx.enter_context(tc.tile_pool(name="v", bufs=2))
        pt_pool = ctx.enter_context(tc.tile_pool(name="pt", bufs=12))
        atu_pool = ctx.enter_context(tc.tile_pool(name="atu", bufs=6))
        at_pool = ctx.enter_context(tc.tile_pool(name="at", bufs=2))
        d_pool = ctx.enter_context(tc.tile_pool(name="d", bufs=8))
        y_pool = ctx.enter_context(tc.tile_pool(name="y", bufs=3))

        ps_mm = ctx.enter_context(tc.tile_pool(name="ps_mm", bufs=3, space="PSUM"))
        ps_s = ctx.enter_context(tc.tile_pool(name="ps_s", bufs=2, space="PSUM"))
        ps_b = ctx.enter_context(tc.tile_pool(name="ps_b", bufs=1, space="PSUM"))
        ps_av = ctx.enter_context(tc.tile_pool(name="ps_av", bufs=2, space="PSUM"))

        # ---- constants / weights (per-contraction-tile tiles so the first
        # matmuls only wait on the first DMA) ----
        wqk = [consts.tile([128, 2 * C], bf16, tag=f"wqk{a}", name=f"wqk{a}") for a in range(CA)]
        wv = [consts.tile([128, C], bf16, tag=f"wv{a}", name=f"wv{a}") for a in range(CA)]
        wp = [consts.tile([128, C], bf16, tag=f"wp{a}", name=f"wp{a}") for a in range(CA)]
        maskT = consts.tile([128, 128], bf16, tag="mask")
        E_sb = consts.tile([2, 128], bf16, tag="E")

        def emit_weight_loads():
            for a in range(CA):
                nc.sync.dma_start(out=wqk[a], in_=wqk_d[128 * a:128 * (a + 1), :])
            for a in range(CA):
                nc.sync.dma_start(out=wv[a], in_=wv_d[128 * a:128 * (a + 1), :])
            for a in range(CA):
                nc.sync.dma_start(out=wp[a], in_=wp_d[128 * a:128 * (a + 1), :])
            nc.sync.dma_start(out=maskT, in_=mask_d[:, :])
            nc.sync.dma_start(out=E_sb, in_=E_d[:, :])
        if have_qkb:
            qkb = consts.tile([128, 12], f32, tag="qkb")
            nc.sync.dma_start(out=qkb, in_=qkb_d[:, :])

        def emit_xt(b):
            xt = [xt_pool.tile([128, N], bf16, tag=f"xt{a}", name=f"xt{a}") for a in range(CA)]
            for a in range(CA):
                nc.sync.dma_start(
                    out=xt[a],
                    in_=xT_d[128 * a:128 * (a + 1), N * b:N * (b + 1)],
                )
            return xt

        def emit_qkv(b, xt):
            """Produce QT/KT [o, t], V [t, o]+ones for batch b."""
            qkt = qkt_pool.tile([128, 12, N], bf16, tag="qkt")
            for ot in range(12):
                for qc in (0, 288):
                    ps = ps_mm.tile([128, 512], f32, tag="mm")
                    for a in range(CA):
                        nc.tensor.matmul(
                            ps[:, :288],
                            lhsT=wqk[a][:, 128 * ot:128 * (ot + 1)],
                            rhs=xt[a][:, qc:qc + 288],
                            start=(a == 0), stop=(a == CA - 1),
                        )
                    if have_qkb:
                        nc.scalar.activation(
                            out=qkt[:, ot, qc:qc + 288], in_=ps[:, :288],
                            func=mybir.ActivationFunctionType.Identity,
                            bias=qkb[:, ot:ot + 1],
                        )
                    else:
                        nc.vector.tensor_copy(out=qkt[:, ot, qc:qc + 288],
                                              in_=ps[:, :288])
            v = v_pool.tile([128, 5, H, HD + 1], bf16, tag="v")
            nc.vector.memset(v[:, :, :, HD:HD + 1], 1.0)
            for kk in range(5):
                tsz = TSZ[kk]
                for ch in range(2):
                    ps = ps_mm.tile([128, 512], f32, tag="mm")
                    for a in range(CA):
                        nc.tensor.matmul(
                            ps[:tsz, :384],
                            lhsT=xt[a][:, 128 * kk:128 * kk + tsz],
                            rhs=wv[a][:, 384 * ch:384 * (ch + 1)],
                            start=(a == 0), stop=(a == CA - 1),
                        )
                    nc.vector.tensor_copy(
                        out=v[0:tsz, kk, 6 * ch:6 * (ch + 1), 0:HD],
                        in_=ps[:tsz, :384],
                    )
            return qkt, v

        def emit_pts(qkt, h, mask_split=False):
            """S^T -> exp -> staircase mask for one head, returns 5 P^T tiles."""
            po = 64 * (h % 2)
            j = h // 2
            QT = qkt[po:po + 64, j, :]
            KT = qkt[po:po + 64, 6 + j, :]
            pts = []
            for kk in range(5):
                ksz = KSZ[kk]
                ko = 128 * kk
                pt = pt_pool.tile([128, N], bf16, tag="pt")
                for (qo, qw) in S_CHUNKS[kk]:
                    ps = ps_s.tile([128, 512], f32, tag="s")
                    nc.tensor.matmul(
                        ps[:ksz, :qw],
                        lhsT=KT[:, ko:ko + ksz],
                        rhs=QT[:, qo:qo + qw],
                        start=True, stop=True,
                    )
                    nc.scalar.activation(
                        out=pt[0:ksz, qo:qo + qw],
                        in_=ps[:ksz, :qw],
                        func=mybir.ActivationFunctionType.Exp,
                        scale=float(SCALE),
                    )
                # staircase mask on the diagonal tile (SBUF-only engines).
                # In the final batch there is no next-batch qkv PE work to
                # hide behind, so split masks across GpSimd and DVE.
                eng = nc.vector if (mask_split and kk % 2) else nc.gpsimd
                eng.tensor_tensor(
                    out=pt[0:ksz, ko:ko + ksz],
                    in0=pt[0:ksz, ko:ko + ksz],
                    in1=maskT[0:ksz, 0:ksz],
                    op=mult,
                )
                pts.append(pt)
            return pts

        def emit_av(v, h, pts, dpair):
            """AV matmuls + unnormalized A^T staging; denom row into dpair."""
            atu = atu_pool.tile([HD + 1, N], f32, tag="atu")
            ps0 = ps_av.tile([HD + 1, 512], f32, tag="av")
            for kk in range(4):
                ko = 128 * kk
                nc.tensor.matmul(
                    ps0[:, ko:512],
                    lhsT=v[0:128, kk, h, :],
                    rhs=pts[kk][0:128, ko:512],
                    start=(kk == 0), stop=(kk == 3),
                )
            nc.vector.tensor_copy(out=atu[:, 0:512], in_=ps0[:, 0:512])
            ps1 = ps_av.tile([HD + 1, 512], f32, tag="av")
            for kk in range(5):
                ksz = KSZ[kk]
                nc.tensor.matmul(
                    ps1[:, 0:64],
                    lhsT=v[0:ksz, kk, h, :],
                    rhs=pts[kk][0:ksz, 512:576],
                    start=(kk == 0), stop=(kk == 4),
                )
            nc.vector.tensor_copy(out=atu[:, 512:576], in_=ps1[:, 0:64])
            nc.sync.dma_start(out=dpair[h % 2:h % 2 + 1, :],
                              in_=atu[HD:HD + 1, :])
            return atu

        def emit_pair_norm(at, j2, dpair, atu0, atu1):
            drecp = d_pool.tile([2, N], f32, tag="drecp")
            nc.vector.reciprocal_approx_fast(out=drecp, in_=dpair)
            drecp16 = d_pool.tile([2, N], bf16, tag="drecp16")
            nc.vector.tensor_copy(out=drecp16, in_=drecp)
            for (qo, qw) in ((0, 512), (512, 64)):
                psb = ps_b.tile([128, 512], f32, tag="b")
                nc.tensor.matmul(
                    psb[:, :qw],
                    lhsT=E_sb[0:2, :],
                    rhs=drecp16[:, qo:qo + qw],
                    start=True, stop=True,
                )
                for hh, atu in ((0, atu0), (1, atu1)):
                    nc.vector.tensor_tensor(
                        out=at[64 * hh:64 * hh + 64, j2, qo:qo + qw],
                        in0=atu[0:64, qo:qo + qw],
                        in1=psb[64 * hh:64 * hh + 64, :qw],
                        op=mult,
                    )

        def emit_attention(b, qkt, v, at):
            """All heads, software-pipelined: S/exp/mask of head h+1 is
            emitted before AV of head h so the PE's in-order stream never
            waits on the ACT->GpSimd chain; pair normalization follows the
            pair's second AV."""
            dpairs = [d_pool.tile([2, N], f32, tag="dpair", name=f"dpair{j}")
                      for j in range(CA)]
            pts_q = {}
            atus = {}
            for h in range(H):
                pts_q[h] = emit_pts(qkt, h, mask_split=(b == BL - 1))
                if h >= 1:
                    atus[h - 1] = emit_av(v, h - 1, pts_q.pop(h - 1),
                                          dpairs[(h - 1) // 2])
                if h >= 2 and h % 2 == 0:
                    j2 = h // 2 - 1
                    emit_pair_norm(at, j2, dpairs[j2],
                                   atus.pop(2 * j2), atus.pop(2 * j2 + 1))
            atus[H - 1] = emit_av(v, H - 1, pts_q.pop(H - 1), dpairs[CA - 1])
            emit_pair_norm(at, CA - 1, dpairs[CA - 1],
                           atus.pop(H - 2), atus.pop(H - 1))

        def emit_proj(b, at):
            for tt in range(5):
                tsz = TSZ[tt]
                to = 128 * tt
                ysb = y_pool.tile([128, C], f32, tag="y")
                for ch in range(2):
                    ps = ps_mm.tile([128, 512], f32, tag="mm")
                    for a in range(CA):
                        nc.tensor.matmul(
                            ps[:tsz, :384],
                            lhsT=at[:, a, to:to + tsz],
                            rhs=wp[a][:, 384 * ch:384 * (ch + 1)],
                            start=(a == 0), stop=(a == CA - 1),
                        )
                    nc.vector.tensor_copy(
                        out=ysb[0:tsz, 384 * ch:384 * (ch + 1)],
                        in_=ps[:tsz, :384],
                    )
                nc.sync.dma_start(
                    out=y_d[N * b + to:N * b + to + tsz, :],
                    in_=ysb[0:tsz, :],
                )

        # software-pipelined batch loop: batch b's producers are emitted
        # before batch b-1's proj so the PE always has dense work. The first
        # batch's x loads are emitted before the bulk of the weights so the
        # first matmul isn't queued behind ~5MB of weight DMA.
        xt = emit_xt(0)
        emit_weight_loads()
        pending = None
        for b in range(BL):
            qkt, v = emit_qkv(b, xt)
            if b + 1 < BL:
                xt = emit_xt(b + 1)
            if pending is not None:
                emit_proj(pending[0], pending[1])
            at = at_pool.tile([128, CA, N], bf16, tag="at")
            emit_attention(b, qkt, v, at)
            pending = (b, at)
        emit_proj(pending[0], pending[1])

    nc.compile()
    return nc


def _get_program(have_qkb=False):
    key = ("nc", have_qkb)
    if key not in _prog_cache:
        _prog_cache[key] = _build_program(have_qkb)
    return _prog_cache[key]


def kernel(x, qkv_w, qkv_b, proj_w, proj_b):
    global LAST_EXEC_NS, LAST_TRACE_PATH
    from concourse.bass_utils import run_bass_kernel_spmd

    x = np.asarray(x, np.float32)
    qkv_w = np.asarray(qkv_w, np.float32)
    qkv_b = np.asarray(qkv_b, np.float32)
    proj_w = np.asarray(proj_w, np.float32)
    proj_b = np.asarray(proj_b, np.float32)

    perm = _block_perm()
    x_s = x[:, perm, :]

    wqkT = np.ascontiguousarray(qkv_w[:2 * C].T).astype(BF16)
    wvT = np.ascontiguousarray(qkv_w[2 * C:].T).astype(BF16)
    wpT = np.ascontiguousarray(proj_w.T).astype(BF16)
    idx = np.arange(128)
    maskT = (idx[:, None] // 16 <= idx[None, :] // 16).astype(BF16)
    # E[i, m] broadcasts drecp16 row i across partitions [64i, 64i+64) when
    # used as lhsT of a K=2 matmul
    E = np.zeros((2, 128), BF16)
    E[0, :64] = 1
    E[1, 64:] = 1

    have_qkb = bool(np.any(qkv_b[:2 * C]))
    shared = {"wqkT": wqkT, "wvT": wvT, "wpT": wpT, "maskT": maskT, "E": E}
    if have_qkb:
        shared["qkb"] = np.ascontiguousarray(
            qkv_b[:2 * C].reshape(12, 128).T).astype(np.float32)
    in_maps = []
    for c in range(CORES):
        xb = x_s[BL * c:BL * (c + 1)]                        # [BL, N, C]
        xT = np.ascontiguousarray(xb.transpose(2, 0, 1).reshape(C, T)).astype(BF16)
        m = {"xT": xT}
        m.update(shared)
        in_maps.append(m)

    nc = _get_program(have_qkb)
    res = run_bass_kernel_spmd(nc, in_maps, core_ids=list(range(CORES)),
                               trace=TRACE)
    LAST_EXEC_NS = res.exec_time_ns
    LAST_TRACE_PATH = (res.instructions_and_trace[1]
                       if res.instructions_and_trace else None)

    y_s = np.empty((B, N, C), np.float32)
    for c in range(CORES):
        y_s[BL * c:BL * (c + 1)] = res.results[c]["y"].reshape(BL, N, C)
    y = np.empty_like(y_s)
    y[:, perm, :] = y_s
    # v-bias and proj-bias contribute a constant per-channel vector to every
    # token: fold them in exactly here (attention rows sum to 1).
    tail = proj_b.astype(np.float64) + qkv_b[2 * C:].astype(np.float64) @ proj_w.T.astype(np.float64)
    if np.any(tail):
        y += tail.astype(np.float32)[None, None, :]
    return y
